# revision 2
# baseline (speedup 1.0000x reference)
"""Trainium2 Bass kernel for nn_ProjectionLayer: mean-pool + projection +
L2-normalize + cosine-sim matrix / pairwise-distance denominator.

Reference math (fp32):
    g = mean(features, axis=2) @ W.T + bias        # [b, out_c]
    g = g / max(||g||_row, 1e-12)                  # L2 normalize rows
    sim = g @ g.T                                  # [b, b]
    dist = ||g + 1e-6||_row                        # [b]
    out = sim / max(dist_i, dist_j, 1e-8)

Sharding: data-parallel over batch (64 rows per core, 8 cores); the
normalized features are AllGather'd (128 KB/rank) so every core can form its
[64, 512] block of the output.
"""

import sys

if "/opt/trn_rl_repo" not in sys.path:
    sys.path.insert(0, "/opt/trn_rl_repo")

import numpy as np

# Problem shapes (hardcoded per contract)
B_FULL = 512     # batch
C_IN = 2048      # in channels (contraction dim of projection)
T_POOL = 196     # pooled (time) dim
O_OUT = 512      # out channels
N_CORES = 8

PD_EPS = 1e-6
NORM_EPS = 1e-12
DENO_EPS = 1e-8


def build_kernel(b_full, c_in, t_pool, o_out, n_cores, bg=4, cpp=4):
    """Emit the Bass module (SPMD program, identical on every core).

    cpp = channels per partition in the feature-pooling layout: partition p of
    chunk k holds channels c = 512k + cpp*p + j (j in [0, cpp)), which makes
    each DMA descriptor a cpp*t_pool*4-byte contiguous run (fewer, bigger
    descriptors -> HWDGE keeps the 16 SDMA engines fed). The channel
    interleave is undone for free by building W^T chunks from stride-cpp
    column slices of W.
    """
    import concourse.mybir as mybir
    import concourse.tile as tile
    from concourse import bacc
    from concourse.masks import make_identity

    f32 = mybir.dt.float32
    AL = mybir.AluOpType

    bc = b_full // n_cores          # batch rows per core
    span = 128 * cpp                # channels per chunk
    nk = c_in // span               # chunks
    nbg = bc // bg                  # feature-tile batch groups
    oc = o_out // 128               # out-channel chunks
    qb = b_full // 128              # gathered-row chunks
    assert bc % bg == 0 and c_in % span == 0 and o_out % 128 == 0
    assert b_full % 128 == 0 and o_out <= 512 and b_full <= 512

    nc = bacc.Bacc("TRN2", target_bir_lowering=False, debug=False,
                   enable_asserts=False, num_devices=n_cores)
    feat = nc.dram_tensor("features", [bc, c_in, t_pool], f32,
                          kind="ExternalInput").ap()
    w_in = nc.dram_tensor("w", [o_out, c_in], f32, kind="ExternalInput").ap()
    bias_in = nc.dram_tensor("bias", [1, o_out], f32, kind="ExternalInput").ap()
    out_d = nc.dram_tensor("out", [bc, b_full], f32, kind="ExternalOutput").ap()

    with tile.TileContext(nc) as tc:
        with (
            tc.tile_pool(name="const", bufs=1) as constp,
            tc.tile_pool(name="wload", bufs=1) as wlp,
            tc.tile_pool(name="wtp", bufs=1) as wtp,
            tc.tile_pool(name="featp", bufs=4) as fp,
            tc.tile_pool(name="lhsp", bufs=1) as lp,
            tc.tile_pool(name="postp", bufs=1) as pp,
            tc.tile_pool(name="psrot", bufs=2, space="PSUM") as psp,
            tc.tile_pool(name="psfix", bufs=1, space="PSUM") as psgp,
            tc.tile_pool(name="dram", bufs=1, space="DRAM") as dp,
        ):
            # ---- constants ----
            ident = constp.tile([128, 128], f32, name="ident")
            make_identity(nc, ident)
            ones = constp.tile([1, bc], f32, name="ones")
            nc.vector.memset(ones, 1.0)
            bias_sb = constp.tile([1, o_out], f32, name="bias_sb")
            nc.sync.dma_start(bias_sb[:], bias_in[:])

            # ---- W^T / t_pool, interleave-matched layout ----
            # wt4[k][j] rows: partition p <-> channel c = span*k + cpp*p + j
            wl = []
            for l in range(oc):
                wli = wlp.tile([128, c_in], f32, name=f"wl{l}")
                nc.sync.dma_start(wli[:], w_in[l * 128:(l + 1) * 128, :])
                wl.append(wli)
            wt4 = []
            for k in range(nk):
                for j in range(cpp):
                    pswt = psp.tile([128, o_out], f32, name="pswt", tag="rot")
                    for l in range(oc):
                        src = wl[l][:, k * span:(k + 1) * span].rearrange(
                            "o (p j) -> o p j", j=cpp)[:, :, j]
                        nc.tensor.transpose(pswt[:, l * 128:(l + 1) * 128],
                                            src, ident[:])
                    wtk = wtp.tile([128, o_out], f32, name=f"wt{k}_{j}")
                    nc.scalar.mul(wtk[:], pswt[:], 1.0 / t_pool)
                    wt4.append(wtk)

            # ---- pooling: p4[k][128p, bc b, cpp j] = sum_t features ----
            p4 = [lp.tile([128, bc, cpp], f32, name=f"p4_{k}") for k in range(nk)]
            idma = 0
            for ibg in range(nbg):
                for k in range(nk):
                    ft = fp.tile([128, bg, cpp, t_pool], f32, name="ft")
                    src = feat[ibg * bg:(ibg + 1) * bg,
                               k * span:(k + 1) * span, :].rearrange(
                                   "b (p j) t -> p b j t", j=cpp)
                    # alternate the two HWDGE rings (SP / ACT) so descriptor
                    # generation is not serialized on one engine
                    dma_eng = nc.sync if idma % 2 == 0 else nc.scalar
                    dma_eng.dma_start(ft[:], src)
                    nc.vector.reduce_sum(p4[k][:, ibg * bg:(ibg + 1) * bg, :],
                                         ft[:], axis=mybir.AxisListType.X)
                    idma += 1

            # ---- projection: g = pooled/t @ W.T + bias  -> PSUM [bc, o_out] ----
            gps = psgp.tile([bc, o_out], f32, name="gps")
            for k in range(nk):
                for j in range(cpp):
                    nc.tensor.matmul(gps[:], p4[k][:, :, j], wt4[k * cpp + j][:],
                                     start=(k == 0 and j == 0), stop=False)
            nc.tensor.matmul(gps[:], ones[:], bias_sb[:], start=False, stop=True)

            # ---- L2 normalize rows ----
            gsb = pp.tile([bc, o_out], f32, name="gsb")
            nc.scalar.copy(gsb[:], gps[:])
            scr = pp.tile([bc, o_out], f32, name="scr")
            nrm2 = pp.tile([bc, 1], f32, name="nrm2")
            nc.vector.tensor_mul(scr[:], gsb[:], gsb[:])
            nc.vector.reduce_sum(nrm2[:], scr[:], axis=mybir.AxisListType.X)
            nrm = pp.tile([bc, 1], f32, name="nrm")
            nc.scalar.sqrt(nrm[:], nrm2[:])
            nmax = pp.tile([bc, 1], f32, name="nmax")
            nc.vector.tensor_scalar_max(nmax[:], nrm[:], NORM_EPS)
            rinv = pp.tile([bc, 1], f32, name="rinv")
            nc.vector.reciprocal(rinv[:], nmax[:])
            gn = pp.tile([bc, o_out], f32, name="gn")
            nc.scalar.mul(gn[:], gsb[:], rinv[:])

            # local dist column: ||gn + eps||_row  [bc, 1]
            nc.vector.tensor_scalar_add(scr[:], gn[:], PD_EPS)
            nc.vector.tensor_mul(scr[:], scr[:], scr[:])
            dl2 = pp.tile([bc, 1], f32, name="dl2")
            nc.vector.reduce_sum(dl2[:], scr[:], axis=mybir.AxisListType.X)
            dl = pp.tile([bc, 1], f32, name="dl")
            nc.scalar.sqrt(dl[:], dl2[:])

            # ---- AllGather normalized features ----
            ag_in = dp.tile([bc, o_out], f32, name="ag_in")
            ag_out = dp.tile([b_full, o_out], f32, name="ag_out",
                             addr_space="Shared")
            nc.sync.dma_start(ag_in[:], gn[:])
            nc.gpsimd.collective_compute(
                "AllGather", AL.bypass,
                replica_groups=[list(range(n_cores))],
                ins=[ag_in.opt()], outs=[ag_out.opt()],
            )

            gf = []
            for q in range(qb):
                gfq = pp.tile([128, o_out], f32, name=f"gf{q}")
                nc.sync.dma_start(gfq[:], ag_out[q * 128:(q + 1) * 128, :])
                gf.append(gfq)

            # dist for all gathered rows: [128, qb]
            scrq = pp.tile([128, o_out], f32, name="scrq")
            d2 = pp.tile([128, qb], f32, name="d2")
            for q in range(qb):
                nc.vector.tensor_scalar_add(scrq[:], gf[q][:], PD_EPS)
                nc.vector.tensor_mul(scrq[:], scrq[:], scrq[:])
                nc.vector.reduce_sum(d2[:, q:q + 1], scrq[:],
                                     axis=mybir.AxisListType.X)
            dist = pp.tile([128, qb], f32, name="dist")
            nc.scalar.sqrt(dist[:], d2[:])

            # dist as a row vector [1, b_full] (PE transpose of columns)
            psdr = psp.tile([1, b_full], f32, name="psdr", tag="rot")
            for q in range(qb):
                nc.tensor.transpose(psdr[:, q * 128:(q + 1) * 128],
                                    dist[:, q:q + 1], ident[:])
            distrow = pp.tile([1, b_full], f32, name="distrow")
            nc.scalar.copy(distrow[:], psdr[:])

            # gathered features transposed: gt[m][128 o, b_full]
            gt = []
            for m in range(oc):
                psgt = psp.tile([128, b_full], f32, name="psgt", tag="rot")
                for q in range(qb):
                    nc.tensor.transpose(psgt[:, q * 128:(q + 1) * 128],
                                        gf[q][:, m * 128:(m + 1) * 128],
                                        ident[:])
                gtm = pp.tile([128, b_full], f32, name=f"gt{m}")
                nc.vector.tensor_copy(gtm[:], psgt[:])
                gt.append(gtm)

            # local rows transposed: gl[m][128 o, bc]
            gl = []
            for m in range(oc):
                psgl = psp.tile([128, bc], f32, name="psgl", tag="rot")
                nc.tensor.transpose(psgl[:], gn[:, m * 128:(m + 1) * 128],
                                    ident[:bc, :bc])
                glm = pp.tile([128, bc], f32, name=f"gl{m}")
                nc.vector.tensor_copy(glm[:], psgl[:])
                gl.append(glm)

            # sim block: [bc, b_full] = gn @ gf.T
            sps = psgp.tile([bc, b_full], f32, name="sps")
            for m in range(oc):
                nc.tensor.matmul(sps[:], gl[m][:], gt[m][:],
                                 start=(m == 0), stop=(m == oc - 1))

            # deno = max(dist_i, dist_j, eps); out = sim / deno
            dps = psgp.tile([bc, b_full], f32, name="dps")
            nc.tensor.matmul(dps[:], ones[:], distrow[:], start=True, stop=True)
            den = pp.tile([bc, b_full], f32, name="den")
            nc.vector.tensor_scalar(den[:], dps[:], dl[:], DENO_EPS,
                                    op0=AL.max, op1=AL.max)
            rden = pp.tile([bc, b_full], f32, name="rden")
            nc.vector.reciprocal(rden[:], den[:])
            outsb = pp.tile([bc, b_full], f32, name="outsb")
            nc.vector.tensor_mul(outsb[:], sps[:], rden[:])
            nc.sync.dma_start(out_d[:], outsb[:])

    nc.compile()
    return nc


_NC_CACHE = {}


def _get_nc():
    key = (B_FULL, C_IN, T_POOL, O_OUT, N_CORES)
    if key not in _NC_CACHE:
        _NC_CACHE[key] = build_kernel(*key)
    return _NC_CACHE[key]


def _run(features, W, bias, trace=False, tmpdir=None):
    from concourse.bass_utils import run_bass_kernel_spmd

    feats = np.ascontiguousarray(np.asarray(features, dtype=np.float32))
    w_np = np.ascontiguousarray(np.asarray(W, dtype=np.float32))
    bias_np = np.ascontiguousarray(
        np.asarray(bias, dtype=np.float32).reshape(1, O_OUT))
    bc = B_FULL // N_CORES

    nc = _get_nc()
    in_maps = [
        {"features": feats[r * bc:(r + 1) * bc], "w": w_np, "bias": bias_np}
        for r in range(N_CORES)
    ]
    kw = {"tmpdir": tmpdir} if tmpdir else {}
    res = run_bass_kernel_spmd(nc, in_maps, core_ids=list(range(N_CORES)),
                               trace=trace, **kw)
    out = np.concatenate([res.results[r]["out"] for r in range(N_CORES)], axis=0)
    return out, res.exec_time_ns


def kernel(features, W, bias):
    out, _ = _run(features, W, bias)
    return out



# revision 3
# speedup vs baseline: 1.1491x; 1.1491x over previous
"""Trainium2 Bass kernel for nn_ProjectionLayer: mean-pool + projection +
L2-normalize + cosine-sim matrix / pairwise-distance denominator.

Reference math (fp32):
    g = mean(features, axis=2) @ W.T + bias        # [b, out_c]
    g = g / max(||g||_row, 1e-12)                  # L2 normalize rows
    sim = g @ g.T                                  # [b, b]
    dist = ||g + 1e-6||_row                        # [b]
    out = sim / max(dist_i, dist_j, 1e-8)

Implementation notes:
  * Since ||g_row|| == 1 after normalization, dist = 1 +- ~3e-6 and
    deno = 1 +- ~3e-6, so out = sim to ~3e-6 relative -- far inside the
    2e-2 gate. The deno computation is dropped.
  * Normalization is scale-invariant, so the kernel projects the raw
    pooled SUM (not mean) and scales bias by t instead; the 1/t never
    needs to be applied.
  * The projection and sim matmuls run in bf16 (pooled sums, W^T, and
    normalized features cast to bf16): ~0.26% rel error, 4x faster PE
    and half-size AllGather.
  * Data-parallel over batch (64 rows/core). The batch is processed in 2
    chunks of 32; each chunk's projection + normalize + transpose +
    AllGather (of the TRANSPOSED normalized features) is issued as soon
    as its pooling finishes, hiding chunk 0's collective under the
    feature streaming of chunk 1.
  * Feature DMA layout: partition p of the per-batch tile holds channels
    16p..16p+15, i.e. one contiguous 12.5KB descriptor per partition per
    1.6MB DMA -- near line-rate HBM streaming.
  * Per-chunk normalize runs on ACT (Square+accum / Copy / Sqrt) and
    GPSIMD (normalize_recip) so the Vector engine never stalls the
    pooling reduces. Collective-adjacent DMAs use SWDGE (gpsimd) so the
    two HWDGE rings stay dedicated to feature streaming.
"""

import sys

if "/opt/trn_rl_repo" not in sys.path:
    sys.path.insert(0, "/opt/trn_rl_repo")

import numpy as np

# Problem shapes (hardcoded per contract)
B_FULL = 512     # batch
C_IN = 2048      # in channels (contraction dim of projection)
T_POOL = 196     # pooled (time) dim
O_OUT = 512      # out channels
N_CORES = 8


def build_kernel(b_full, c_in, t_pool, o_out, n_cores, n_chunks=2, ft_bufs=8):
    import concourse.mybir as mybir
    import concourse.tile as tile
    from concourse import bacc
    from concourse.masks import make_identity

    f32 = mybir.dt.float32
    bf16 = mybir.dt.bfloat16
    AL = mybir.AluOpType
    AF = mybir.ActivationFunctionType
    X = mybir.AxisListType.X

    bc = b_full // n_cores          # batch rows per core (64)
    nj = 16                         # channels per partition (c = 16p + j)
    ck = bc // n_chunks             # rows per chunk
    oc = o_out // 128               # out-channel 128-blocks (4)
    nr = n_cores
    assert c_in == 128 * nj and bc % n_chunks == 0 and o_out % 128 == 0

    nc = bacc.Bacc("TRN2", target_bir_lowering=False, debug=False,
                   enable_asserts=False, num_devices=n_cores)
    feat = nc.dram_tensor("features", [bc, c_in, t_pool], f32,
                          kind="ExternalInput").ap()
    w_in = nc.dram_tensor("w", [o_out, c_in], f32, kind="ExternalInput").ap()
    bias_in = nc.dram_tensor("bias", [1, o_out], f32, kind="ExternalInput").ap()
    out_d = nc.dram_tensor("out", [bc, b_full], f32, kind="ExternalOutput").ap()

    with tile.TileContext(nc) as tc:
        with (
            tc.tile_pool(name="const", bufs=1) as constp,
            tc.tile_pool(name="wload", bufs=1) as wlp,
            tc.tile_pool(name="wtp", bufs=1) as wtp,
            tc.tile_pool(name="featp", bufs=ft_bufs) as fp,
            tc.tile_pool(name="poolp", bufs=1) as lp,
            tc.tile_pool(name="postp", bufs=1) as pp,
            tc.tile_pool(name="psrot", bufs=2, space="PSUM") as psp,
            tc.tile_pool(name="psgps", bufs=2, space="PSUM") as psgp,
            tc.tile_pool(name="pssim", bufs=2, space="PSUM") as pssp,
            tc.tile_pool(name="dram", bufs=1, space="DRAM") as dp,
        ):
            # ---- constants ----
            identf = constp.tile([128, 128], f32, name="identf")
            make_identity(nc, identf)
            identb = constp.tile([ck, ck], bf16, name="identb")
            make_identity(nc, identb)
            ones = constp.tile([1, ck], bf16, name="ones")
            nc.vector.memset(ones, 1.0)
            bias_sb = constp.tile([1, o_out], f32, name="bias_sb")
            nc.sync.dma_start(bias_sb[:], bias_in[:])
            # normalization is scale-invariant: project the pooled SUM and
            # scale bias by t_pool instead of dividing the sum by t_pool
            bias_t = constp.tile([1, o_out], bf16, name="bias_t")
            nc.scalar.mul(bias_t[:], bias_sb[:], float(t_pool))

            # ---- W^T in bf16, interleave-matched layout ----
            # wt[j] rows: partition p <-> channel c = nj*p + j
            wl = []
            for l in range(oc):
                wli = wlp.tile([128, c_in], f32, name=f"wl{l}")
                nc.sync.dma_start(wli[:], w_in[l * 128:(l + 1) * 128, :])
                wl.append(wli)
            wt = []
            for j in range(nj):
                pswt = psp.tile([128, o_out], f32, name="pswt", tag="rot")
                for l in range(oc):
                    src = wl[l][:, :].rearrange("o (p j) -> o p j", j=nj)[:, :, j]
                    nc.tensor.transpose(pswt[:, l * 128:(l + 1) * 128],
                                        src, identf[:])
                wtj = wtp.tile([128, o_out], bf16, name=f"wt{j}")
                nc.scalar.copy(wtj[:], pswt[:])
                wt.append(wtj)

            # ---- per-chunk state ----
            gl_full = pp.tile([128, oc, bc], bf16, name="gl_full")
            outsb = pp.tile([bc, b_full], f32, name="outsb")
            grts = []

            for c in range(n_chunks):
                # ---- pooling for this chunk: p4[128p, i, j] = sum_t ----
                p4 = lp.tile([128, ck, nj], bf16, name=f"p4_{c}")
                for i in range(ck):
                    b = c * ck + i
                    ft = fp.tile([128, nj * t_pool], f32, name="ft")
                    src = feat[b:b + 1, :, :].rearrange(
                        "b (p j) t -> p (b j t)", j=nj)
                    dma_eng = nc.scalar if b % 2 == 0 else nc.sync
                    dma_eng.dma_start(ft[:], src)
                    with nc.allow_low_precision("pooled sums cast to bf16"):
                        nc.vector.reduce_sum(
                            p4[:, i, :],
                            ft[:].rearrange("p (j t) -> p j t", t=t_pool),
                            axis=X)

                # ---- projection: gps = pooled_sum @ W.T + t*bias ----
                gps = psgp.tile([ck, o_out], f32, name="gps", tag="gps")
                for j in range(nj):
                    nc.tensor.matmul(gps[:], p4[:, :, j], wt[j][:],
                                     start=(j == 0), stop=False)
                nc.tensor.matmul(gps[:], ones[:], bias_t[:],
                                 start=False, stop=True)

                # ---- L2 normalize rows (ACT + GPSIMD; DVE stays free) ----
                scr = pp.tile([ck, o_out], f32, name=f"scr{c}")
                n2 = pp.tile([ck, 1], f32, name=f"n2{c}")
                nc.scalar.activation(scr[:], gps[:], AF.Square, accum_out=n2[:])
                gsb = pp.tile([ck, o_out], f32, name=f"gsb{c}")
                nc.scalar.copy(gsb[:], gps[:])
                nrm = pp.tile([ck, 1], f32, name=f"nrm{c}")
                nc.scalar.sqrt(nrm[:], n2[:])
                gn = pp.tile([ck, o_out], bf16, name=f"gn{c}")
                nc.gpsimd.normalize_recip(gn[:], gsb[:], nrm[:])

                # ---- transpose gn -> [o, b] blocks (PE), stash local+ship ----
                glc = pp.tile([128, oc, ck], bf16, name=f"glc{c}")
                for m in range(oc):
                    psg = psp.tile([128, ck], bf16, name="psg", tag="rot")
                    nc.tensor.transpose(psg[:], gn[:, m * 128:(m + 1) * 128],
                                        identb[:])
                    nc.scalar.copy(gl_full[:, m, c * ck:(c + 1) * ck], psg[:])
                    nc.scalar.copy(glc[:, m, :], psg[:])

                # ---- AllGather transposed features (SWDGE for all DMAs) ----
                agin = dp.tile([128, oc * ck], bf16, name=f"agin{c}")
                agout = dp.tile([nr * 128, oc * ck], bf16, name=f"agout{c}",
                                addr_space="Shared")
                nc.gpsimd.dma_start(agin[:], glc[:])
                nc.gpsimd.collective_compute(
                    "AllGather", AL.bypass,
                    replica_groups=[list(range(n_cores))],
                    ins=[agin.opt()], outs=[agout.opt()],
                )
                grt = pp.tile([128, nr, oc * ck], bf16, name=f"grt{c}")
                nc.gpsimd.dma_start(
                    grt[:], agout[:, :].rearrange("(r p) f -> p r f", r=nr))
                grts.append(grt)

            # ---- sim blocks: out[:, j] for j = r*bc + c*ck + i ----
            for c in range(n_chunks):
                simps = pssp.tile([bc, nr * ck], f32, name="simps", tag="sim")
                for m in range(oc):
                    nc.tensor.matmul(
                        simps[:], gl_full[:, m, :],
                        grts[c][:, :, m * ck:(m + 1) * ck],
                        start=(m == 0), stop=(m == oc - 1))
                dst = outsb[:, :].rearrange(
                    "b (r c i) -> b r c i", c=n_chunks, i=ck)[:, :, c, :]
                nc.vector.tensor_copy(dst, simps[:])

            nc.sync.dma_start(out_d[:], outsb[:])

    nc.compile()
    return nc


_NC_CACHE = {}


def _get_nc():
    key = (B_FULL, C_IN, T_POOL, O_OUT, N_CORES)
    if key not in _NC_CACHE:
        _NC_CACHE[key] = build_kernel(*key)
    return _NC_CACHE[key]


def _run(features, W, bias, trace=False, tmpdir=None):
    from concourse.bass_utils import run_bass_kernel_spmd

    feats = np.ascontiguousarray(np.asarray(features, dtype=np.float32))
    w_np = np.ascontiguousarray(np.asarray(W, dtype=np.float32))
    bias_np = np.ascontiguousarray(
        np.asarray(bias, dtype=np.float32).reshape(1, O_OUT))
    bc = B_FULL // N_CORES

    nc = _get_nc()
    in_maps = [
        {"features": feats[r * bc:(r + 1) * bc], "w": w_np, "bias": bias_np}
        for r in range(N_CORES)
    ]
    kw = {"tmpdir": tmpdir} if tmpdir else {}
    res = run_bass_kernel_spmd(nc, in_maps, core_ids=list(range(N_CORES)),
                               trace=trace, **kw)
    out = np.concatenate([res.results[r]["out"] for r in range(N_CORES)], axis=0)
    return out, res.exec_time_ns


def kernel(features, W, bias):
    out, _ = _run(features, W, bias)
    return out


# revision 12
# speedup vs baseline: 1.1633x; 1.0123x over previous
"""Fallback: chunked bf16 kernel with ncfw AllGathers (r1) + tail trims.

Same as the 385us r1 kernel, plus: chunk-1 normalize on the fast ACT+DVE
path (stream is over, DVE is idle), and chunk-1 collective-adjacent DMAs
on the sync HWDGE ring (free after streaming ends, lower fixed cost than
SWDGE)."""

import sys

if "/opt/trn_rl_repo" not in sys.path:
    sys.path.insert(0, "/opt/trn_rl_repo")

import numpy as np

B_FULL = 512
C_IN = 2048
T_POOL = 196
O_OUT = 512
N_CORES = 8

N_CHUNKS = 2


def build_kernel(b_full, c_in, t_pool, o_out, n_cores, ft_bufs=8):
    import concourse.mybir as mybir
    import concourse.tile as tile
    from concourse import bacc
    from concourse.masks import make_identity

    f32 = mybir.dt.float32
    bf16 = mybir.dt.bfloat16
    AL = mybir.AluOpType
    AF = mybir.ActivationFunctionType
    X = mybir.AxisListType.X

    bc = b_full // n_cores
    nj = 16
    ck = bc // N_CHUNKS
    oc = o_out // 128
    nr = n_cores
    assert c_in == 128 * nj and bc % N_CHUNKS == 0 and o_out % 128 == 0

    nc = bacc.Bacc("TRN2", target_bir_lowering=False, debug=False,
                   enable_asserts=False, num_devices=n_cores)
    feat = nc.dram_tensor("features", [bc, c_in, t_pool], f32,
                          kind="ExternalInput").ap()
    w_in = nc.dram_tensor("w", [o_out, c_in], f32, kind="ExternalInput").ap()
    bias_in = nc.dram_tensor("bias", [1, o_out], f32, kind="ExternalInput").ap()
    out_d = nc.dram_tensor("out", [bc, b_full], f32, kind="ExternalOutput").ap()

    with tile.TileContext(nc) as tc:
        with (
            tc.tile_pool(name="const", bufs=1) as constp,
            tc.tile_pool(name="wload", bufs=1) as wlp,
            tc.tile_pool(name="wtp", bufs=1) as wtp,
            tc.tile_pool(name="featp", bufs=ft_bufs) as fp,
            tc.tile_pool(name="poolp", bufs=1) as lp,
            tc.tile_pool(name="postp", bufs=1) as pp,
            tc.tile_pool(name="psrot", bufs=2, space="PSUM") as psp,
            tc.tile_pool(name="psgps", bufs=2, space="PSUM") as psgp,
            tc.tile_pool(name="pssim", bufs=2, space="PSUM") as pssp,
            tc.tile_pool(name="dram", bufs=1, space="DRAM") as dp,
        ):
            # ---- constants ----
            identf = constp.tile([128, 128], f32, name="identf")
            make_identity(nc, identf)
            identb = constp.tile([ck, ck], bf16, name="identb")
            make_identity(nc, identb)
            ones = constp.tile([1, ck], bf16, name="ones")
            nc.vector.memset(ones, 1.0)
            bias_sb = constp.tile([1, o_out], f32, name="bias_sb")
            nc.sync.dma_start(bias_sb[:], bias_in[:])
            bias_t = constp.tile([1, o_out], bf16, name="bias_t")
            nc.scalar.mul(bias_t[:], bias_sb[:], float(t_pool))

            # ---- W^T in bf16 ----
            wl = []
            for l in range(oc):
                wli = wlp.tile([128, c_in], f32, name=f"wl{l}")
                nc.sync.dma_start(wli[:], w_in[l * 128:(l + 1) * 128, :])
                wl.append(wli)
            wt = []
            for j in range(nj):
                pswt = psp.tile([128, o_out], f32, name="pswt", tag="rot")
                for l in range(oc):
                    src = wl[l][:, :].rearrange("o (p j) -> o p j", j=nj)[:, :, j]
                    nc.tensor.transpose(pswt[:, l * 128:(l + 1) * 128],
                                        src, identf[:])
                wtj = wtp.tile([128, o_out], bf16, name=f"wt{j}")
                nc.scalar.copy(wtj[:], pswt[:])
                wt.append(wtj)

            gl_full = pp.tile([128, oc, bc], bf16, name="gl_full")
            outsb = pp.tile([bc, b_full], f32, name="outsb")
            glcs = [pp.tile([128, oc * ck], bf16, name=f"glc{c}")
                    for c in range(N_CHUNKS)]
            grts = []

            def pool_chunk(c):
                p4 = lp.tile([128, ck, nj], bf16, name=f"p4_{c}")
                for i in range(ck):
                    b = c * ck + i
                    ft = fp.tile([128, nj * t_pool], f32, name="ft")
                    src = feat[b:b + 1, :, :].rearrange(
                        "b (p j) t -> p (b j t)", j=nj)
                    dma_eng = nc.scalar if b % 2 == 0 else nc.sync
                    dma_eng.dma_start(ft[:], src)
                    with nc.allow_low_precision("pooled sums cast to bf16"):
                        nc.vector.reduce_sum(
                            p4[:, i, :],
                            ft[:].rearrange("p (j t) -> p j t", t=t_pool),
                            axis=X)
                return p4

            def project(c, p4):
                gps = psgp.tile([ck, o_out], f32, name="gps", tag="gps")
                for j in range(nj):
                    nc.tensor.matmul(gps[:], p4[:, :, j], wt[j][:],
                                     start=(j == 0), stop=False)
                nc.tensor.matmul(gps[:], ones[:], bias_t[:],
                                 start=False, stop=True)
                return gps

            def transpose_gn(c, gn):
                glc_v = glcs[c][:].rearrange("p (m i) -> p m i", i=ck)
                for m in range(oc):
                    psg = psp.tile([128, ck], bf16, name="psg", tag="rot")
                    nc.tensor.transpose(psg[:], gn[:, m * 128:(m + 1) * 128],
                                        identb[:])
                    nc.scalar.copy(gl_full[:, m, c * ck:(c + 1) * ck], psg[:])
                    nc.scalar.copy(glc_v[:, m, :], psg[:])

            def allgather(c, dma_eng):
                agin = dp.tile([128, oc * ck], bf16, name=f"agin{c}")
                agout = dp.tile([nr * 128, oc * ck], bf16, name=f"agout{c}",
                                addr_space="Shared")
                dma_eng.dma_start(agin[:], glcs[c][:])
                nc.gpsimd.collective_compute(
                    "AllGather", AL.bypass,
                    replica_groups=[list(range(n_cores))],
                    ins=[agin.opt()], outs=[agout.opt()],
                )
                grt = pp.tile([128, nr, oc * ck], bf16, name=f"grt{c}")
                dma_eng.dma_start(
                    grt[:], agout[:, :].rearrange("(r p) f -> p r f", r=nr))
                grts.append(grt)

            # ================= chunk 0 =================
            p4 = pool_chunk(0)
            gps = project(0, p4)
            scr = pp.tile([ck, o_out], f32, name="scr0")
            n2 = pp.tile([ck, 1], f32, name="n20")
            nc.scalar.activation(scr[:], gps[:], AF.Square, accum_out=n2[:])
            gsb = pp.tile([ck, o_out], f32, name="gsb0")
            nc.scalar.copy(gsb[:], gps[:])
            nrm = pp.tile([ck, 1], f32, name="nrm0")
            nc.scalar.sqrt(nrm[:], n2[:])
            gn0 = pp.tile([ck, o_out], bf16, name="gn0")
            nc.gpsimd.normalize_recip(gn0[:], gsb[:], nrm[:])
            transpose_gn(0, gn0)
            allgather(0, nc.gpsimd)

            # ================= chunk 1 =================
            p4 = pool_chunk(1)
            gps = project(1, p4)
            scr1 = pp.tile([ck, o_out], f32, name="scr1")
            n21 = pp.tile([ck, 1], f32, name="n21")
            nc.scalar.activation(scr1[:], gps[:], AF.Square, accum_out=n21[:])
            nrm1 = pp.tile([ck, 1], f32, name="nrm1")
            nc.scalar.sqrt(nrm1[:], n21[:])
            rinv1 = pp.tile([ck, 1], f32, name="rinv1")
            nc.vector.reciprocal(rinv1[:], nrm1[:])
            gn1 = pp.tile([ck, o_out], bf16, name="gn1")
            nc.scalar.mul(gn1[:], gps[:], rinv1[:])
            transpose_gn(1, gn1)
            allgather(1, nc.sync)

            for c in range(N_CHUNKS):
                simps = pssp.tile([bc, nr * ck], f32, name=f"simps{c}",
                                  tag="sim")
                for m in range(oc):
                    nc.tensor.matmul(
                        simps[:], gl_full[:, m, :],
                        grts[c][:, :, m * ck:(m + 1) * ck],
                        start=(m == 0), stop=(m == oc - 1))
                dst = outsb[:, :].rearrange(
                    "b (r c i) -> b r c i", c=N_CHUNKS, i=ck)[:, :, c, :]
                nc.vector.tensor_copy(dst, simps[:])

            nc.sync.dma_start(out_d[:], outsb[:])

    nc.compile()
    return nc


_NC_CACHE = {}


def _get_nc():
    key = (B_FULL, C_IN, T_POOL, O_OUT, N_CORES)
    if key not in _NC_CACHE:
        _NC_CACHE[key] = build_kernel(*key)
    return _NC_CACHE[key]


def _run(features, W, bias, trace=False, tmpdir=None):
    from concourse.bass_utils import run_bass_kernel_spmd

    feats = np.ascontiguousarray(np.asarray(features, dtype=np.float32))
    w_np = np.ascontiguousarray(np.asarray(W, dtype=np.float32))
    bias_np = np.ascontiguousarray(
        np.asarray(bias, dtype=np.float32).reshape(1, O_OUT))
    bc = B_FULL // N_CORES

    nc = _get_nc()
    in_maps = [
        {"features": feats[r * bc:(r + 1) * bc], "w": w_np, "bias": bias_np}
        for r in range(N_CORES)
    ]
    kw = {"tmpdir": tmpdir} if tmpdir else {}
    res = run_bass_kernel_spmd(nc, in_maps, core_ids=list(range(N_CORES)),
                               trace=trace, **kw)
    out = np.concatenate([res.results[r]["out"] for r in range(N_CORES)], axis=0)
    return out, res.exec_time_ns


def kernel(features, W, bias):
    out, _ = _run(features, W, bias)
    return out
